# revision 6
# baseline (speedup 1.0000x reference)
"""MoE layer (top-2 of 8 experts) on 8 TRN2 NeuronCores, expert-parallel.

Host side: router (exact replica of the reference jax ops, so top-k
selection bit-matches), token gather by expert assignment, weight
repacking into DMA-friendly layouts, and the final weighted scatter-add.

Device side (one expert per core, SPMD): the full expert FFN
    h = X @ W1 ; act = gelu(h_gate) * h_up ; Y = act @ W2
computed in bf16 (fp32 PSUM accumulation, ~4e-3 rel err) with all
activations kept transposed (tokens on the free axis) so no on-device
transposes are needed.

Capacity-factor-1.0 expert parallelism: each core processes up to
C = N*TOP_K/n_cores tokens of its expert (two exact 512-column matmul
chunks); the <1% overflow tokens of over-subscribed experts are
computed exactly on host (fp32 BLAS) and merged in the combine step.

Self-contained: only library imports (numpy/jax/ml_dtypes/concourse),
no file reads.
"""

import numpy as np
import ml_dtypes

BF16 = ml_dtypes.bfloat16
TOP_K = 2
EPS = 1e-6
P = 128
D = 2048
F = 2048  # expert hidden dim (ED)
E = 8
KO = D // P  # 16 K-tiles for matmul1 / output D-tiles
MJ = F // P  # 16 gate/up tile pairs; also K-tiles for matmul2

_BUILD_CACHE: dict = {}

# Activation for the gate branch. CoreSim doesn't implement Gelu, so tests
# can set this to "Identity" for structural sim validation.
ACT_FN = "Gelu"


def _chunks_of(C: int) -> list[tuple[int, int]]:
    """Split the token-capacity free axis into matmul chunks <= 512."""
    if C <= 512:
        return [(0, C)]
    nch = -(-C // 512)
    base = C // nch
    base -= base % 8
    sizes = [base] * nch
    rem = C - base * nch
    i = 0
    while rem > 0:
        add = min(8, rem)
        sizes[i % nch] += add
        rem -= add
        i += 1
    out = []
    off = 0
    for s in sizes:
        out.append((off, s))
        off += s
    assert off == C
    return out


def _build(C: int):
    """Build + compile the per-core expert-FFN bass program for capacity C."""
    key = (C, ACT_FN)
    if key in _BUILD_CACHE:
        return _BUILD_CACHE[key]

    import concourse.bacc as bacc
    import concourse.mybir as mybir
    import concourse.tile as tile
    f32 = mybir.dt.float32
    bf16 = mybir.dt.bfloat16
    act_fn = getattr(mybir.ActivationFunctionType, ACT_FN)
    chunks = _chunks_of(C)

    nc = bacc.Bacc(
        "TRN2", target_bir_lowering=False, debug=False, enable_asserts=False
    )
    # Packed layouts (host pre-transposed, partition-major):
    #   xt[p, ko, c]    = X^T[ko*128+p, c]          (tokens on free axis)
    #   w1[p, m, ko, q] = W1perm[ko*128+p, m*128+q] (m: g0,u0,g1,u1,... strips)
    #   w2[p, i, fo, q] = W2[fo*128+p, i*128+q]
    #   yt[p, io, c]    = Y^T[io*128+p, c]
    xt_d = nc.dram_tensor("xt", [P, KO, C], bf16, kind="ExternalInput")
    w1_d = nc.dram_tensor("w1", [P, 2 * MJ, KO, P], bf16, kind="ExternalInput")
    w2_d = nc.dram_tensor("w2", [P, KO, MJ, P], bf16, kind="ExternalInput")
    yt_d = nc.dram_tensor("yt", [P, KO, C], bf16, kind="ExternalOutput")

    NWARM = 22  # PE warm-up matmuls covering the DMA-latency head (~9us)

    with tile.TileContext(nc) as tc:
        with (
            tc.tile_pool(name="xt", bufs=1) as xt_pool,
            tc.tile_pool(name="act", bufs=1) as act_pool,
            tc.tile_pool(name="wu", bufs=1) as wu_pool,
            tc.tile_pool(name="w1", bufs=8) as w1_pool,
            tc.tile_pool(name="w2", bufs=2) as w2_pool,
            tc.tile_pool(name="tg", bufs=3) as tg_pool,
            tc.tile_pool(name="yo", bufs=3) as yo_pool,
            tc.tile_pool(name="ps", bufs=8, space="PSUM") as ps_pool,
        ):
            nch = len(chunks)

            # PE warm-up: dummy matmuls on memset data, issued ahead of the
            # real stream. They run while the head DMAs are in flight, so the
            # PE is at full clock (and HAM at K=8/8) when real data lands,
            # instead of paying the 1.2 GHz ramp + idle-triggered throttle.
            wz = wu_pool.tile([P, P], bf16)
            wx = wu_pool.tile([P, 512], bf16)
            nc.vector.memset(wz[:], 0.0)
            nc.vector.memset(wx[:], 0.0)
            pw = ps_pool.tile([P, 512], f32, tag="ps")
            for _ in range(NWARM):
                nc.tensor.matmul(pw[:], wz[:], wx[:], start=True, stop=True)

            # Head DMA plan, in PE demand order, x spread across both HWDGE
            # rings. The ffn1 unit order below keeps the first three j's on
            # chunk 0 only, so chunk 1 of x isn't needed until ~33us in.
            w1_tiles = {}

            def issue_w1(m, ring):
                t = w1_pool.tile([P, KO, P], bf16, tag="w1s")
                ring.dma_start(t[:], w1_d.ap()[:, m])
                w1_tiles[m] = t

            xt_sb = xt_pool.tile([P, KO, C], bf16)

            def issue_xt(ring, ci, k0, k1):
                c0, cn = chunks[ci]
                ring.dma_start(
                    xt_sb[:, k0:k1, c0 : c0 + cn], xt_d.ap()[:, k0:k1, c0 : c0 + cn]
                )

            t = w1_pool.tile([P, KO, P], bf16, tag="w1s")
            nc.sync.dma_start(t[:, : KO // 2], w1_d.ap()[:, 0, : KO // 2])
            issue_xt(nc.sync, 0, 0, 4)
            issue_xt(nc.scalar, 0, 4, 8)
            nc.sync.dma_start(t[:, KO // 2 :], w1_d.ap()[:, 0, KO // 2 :])
            w1_tiles[0] = t
            issue_xt(nc.scalar, 0, 8, 12)
            t = w1_pool.tile([P, KO, P], bf16, tag="w1s")
            nc.sync.dma_start(t[:, : KO // 2], w1_d.ap()[:, 1, : KO // 2])
            issue_xt(nc.scalar, 0, 12, 16)
            nc.sync.dma_start(t[:, KO // 2 :], w1_d.ap()[:, 1, KO // 2 :])
            w1_tiles[1] = t
            issue_w1(2, nc.sync)
            issue_w1(3, nc.scalar)
            issue_w1(4, nc.scalar)
            issue_w1(5, nc.scalar)
            if nch >= 2:
                issue_xt(nc.sync, 1, 0, 4)
                issue_xt(nc.sync, 1, 4, 8)
                issue_xt(nc.scalar, 1, 8, 12)
                issue_xt(nc.scalar, 1, 12, 16)
            for ci in range(2, nch):
                for k0 in range(0, KO, 4):
                    issue_xt(nc.scalar, ci, k0, k0 + 4)
            act_sb = act_pool.tile([P, MJ, C], bf16)

            # ffn1 unit order: chunk-major over the first three (preloaded)
            # j's so the early PE demand matches the head DMA supply, then
            # j-major for the rest.
            head_js = min(3, MJ)
            order = [(j, ci) for ci in range(nch) for j in range(head_js)]
            order += [(j, ci) for j in range(head_js, MJ) for ci in range(nch)]
            last_unit = {}
            for idx, (j, ci) in enumerate(order):
                last_unit[j] = idx

            def get_w1(m):
                if m not in w1_tiles:
                    t = w1_pool.tile([P, KO, P], bf16, tag="w1s")
                    nc.scalar.dma_start(t[:], w1_d.ap()[:, m])
                    w1_tiles[m] = t
                return w1_tiles[m]

            with nc.named_scope("ffn1"):
                for idx, (j, ci) in enumerate(order):
                    wg = get_w1(2 * j)
                    wu = get_w1(2 * j + 1)
                    c0, cn = chunks[ci]
                    pg = ps_pool.tile([P, 512], f32, tag="ps")
                    pu = ps_pool.tile([P, 512], f32, tag="ps")
                    for ko in range(KO):
                        nc.tensor.matmul(
                            pg[:, :cn],
                            wg[:, ko],
                            xt_sb[:, ko, c0 : c0 + cn],
                            start=(ko == 0),
                            stop=(ko == KO - 1),
                        )
                    for ko in range(KO):
                        nc.tensor.matmul(
                            pu[:, :cn],
                            wu[:, ko],
                            xt_sb[:, ko, c0 : c0 + cn],
                            start=(ko == 0),
                            stop=(ko == KO - 1),
                        )
                    tg = tg_pool.tile([P, 512], f32, tag="tg")
                    nc.scalar.activation(tg[:, :cn], pg[:, :cn], act_fn)
                    nc.vector.tensor_mul(
                        out=act_sb[:, j, c0 : c0 + cn],
                        in0=tg[:, :cn],
                        in1=pu[:, :cn],
                    )
                    if idx == last_unit[j]:
                        w1_tiles.pop(2 * j, None)
                        w1_tiles.pop(2 * j + 1, None)

            with nc.named_scope("ffn2"):
                for i in range(KO):
                    w2t = w2_pool.tile([P, MJ, P], bf16, tag="w2s")
                    (nc.sync if i < 4 else nc.scalar).dma_start(
                        w2t[:], w2_d.ap()[:, i]
                    )
                    units = list(chunks)
                    if i == KO - 1 and units[-1][1] >= 256:
                        c0, cn = units.pop()
                        h = (cn // 2 + 7) // 8 * 8
                        q = (cn - h) // 2 // 8 * 8
                        units += [(c0, h), (c0 + h, cn - h - q), (c0 + cn - q, q)]
                    for c0, cn in units:
                        py = ps_pool.tile([P, 512], f32, tag="ps")
                        for fo in range(MJ):
                            nc.tensor.matmul(
                                py[:, :cn],
                                w2t[:, fo],
                                act_sb[:, fo, c0 : c0 + cn],
                                start=(fo == 0),
                                stop=(fo == MJ - 1),
                            )
                        yo = yo_pool.tile([P, 512], bf16, tag="yo")
                        nc.vector.tensor_copy(out=yo[:, :cn], in_=py[:, :cn])
                        nc.sync.dma_start(yt_d.ap()[:, i, c0 : c0 + cn], yo[:, :cn])

    nc.compile()
    _BUILD_CACHE[key] = nc
    return nc


def _router(x, router_scale, gate_w):
    """Replicate the reference router ops exactly (same jax ops, default
    backend) so the top-2 expert selection bit-matches the reference."""
    import jax
    import jax.numpy as jnp

    x = jnp.asarray(x)
    router_scale = jnp.asarray(router_scale)
    gate_w = jnp.asarray(gate_w)
    _B, _L, d = x.shape
    h = x * jax.lax.rsqrt(jnp.mean(x * x, axis=-1, keepdims=True) + EPS)
    h = h * (d**-0.5) * router_scale
    logits = (h @ gate_w).astype(jnp.float32)
    probs = jax.nn.softmax(logits, axis=-1)
    w, idx = jax.lax.top_k(probs, TOP_K)
    w = w / jnp.clip(jnp.sum(w, axis=-1, keepdims=True), 1e-12)
    w = w.astype(x.dtype)
    return (
        np.asarray(idx).reshape(-1, TOP_K),
        np.asarray(w).reshape(-1, TOP_K).astype(np.float32),
    )


def _gelu_exact(v: np.ndarray) -> np.ndarray:
    """gelu(approximate=False) = v * Phi(v), matching jax.nn.gelu."""
    try:
        from scipy.special import erf
    except Exception:  # pragma: no cover - scipy ships with jax
        import math

        erf = np.vectorize(math.erf, otypes=[np.float64])
    return (v * 0.5 * (1.0 + erf(v / np.sqrt(2.0)))).astype(np.float32)


def _host_ffn(xe: np.ndarray, gate_up_e: np.ndarray, down_e: np.ndarray):
    """Exact fp32 expert FFN for the few capacity-overflow tokens."""
    h = xe @ gate_up_e
    act = _gelu_exact(h[:, :F]) * h[:, F:]
    return act @ down_e


def _pack_w1(gate_up_e: np.ndarray) -> np.ndarray:
    """[D, 2F] -> [P, 2*MJ, KO, P] with gate/up 128-col strips interleaved."""
    g = gate_up_e[:, :F].reshape(D, MJ, P)
    u = gate_up_e[:, F:].reshape(D, MJ, P)
    w1p = np.empty((D, 2 * MJ, P), np.float32)
    w1p[:, 0::2] = g
    w1p[:, 1::2] = u
    # [D, 2MJ, P] -> [KO, P, 2MJ, P] -> [P, 2MJ, KO, P]
    return np.ascontiguousarray(
        w1p.reshape(KO, P, 2 * MJ, P).transpose(1, 2, 0, 3).astype(BF16)
    )


def _pack_w2(down_e: np.ndarray) -> np.ndarray:
    """[F, D] -> [P, KO, MJ, P]  (w2[p, i, fo, q] = W2[fo*128+p, i*128+q])."""
    return np.ascontiguousarray(
        down_e.reshape(MJ, P, KO, P).transpose(1, 2, 0, 3).astype(BF16)
    )


def run_moe(x, router_scale, gate_w, gate_up, down, per_expert_scale, trace=False):
    from concourse import bass_utils

    x = np.asarray(x, dtype=np.float32)
    router_scale = np.asarray(router_scale, dtype=np.float32)
    gate_w = np.asarray(gate_w, dtype=np.float32)
    gate_up = np.asarray(gate_up, dtype=np.float32)
    down = np.asarray(down, dtype=np.float32)
    per_expert_scale = np.asarray(per_expert_scale, dtype=np.float32)

    B, L, d = x.shape
    N = B * L
    assert d == D and gate_up.shape == (E, D, 2 * F) and down.shape == (E, F, D)

    idxf, wf = _router(x, router_scale, gate_w)

    pair_expert = idxf.reshape(-1)
    pair_token = np.repeat(np.arange(N), TOP_K)
    pair_w = wf.reshape(-1) * per_expert_scale[pair_expert]

    order = np.argsort(pair_expert, kind="stable")
    tok_o = pair_token[order]
    w_o = pair_w[order]
    counts = np.bincount(pair_expert, minlength=E)
    offs = np.zeros(E + 1, np.int64)
    offs[1:] = np.cumsum(counts)

    # Capacity factor 1.0: each core takes up to C tokens of its expert;
    # the <1% overflow of over-subscribed experts is computed exactly on
    # host and merged below.
    CMAX = max(512, -(-(N * TOP_K // E) // 8) * 8)
    C = max(64, min(CMAX, -(-int(counts.max()) // 8) * 8))

    nc = _build(C)

    xf = x.reshape(N, D)
    xf16 = xf.astype(BF16)

    in_maps = []
    dev_n = []
    for e in range(E):
        lo, hi = offs[e], offs[e + 1]
        n_dev = min(C, hi - lo)
        dev_n.append(n_dev)
        toks = tok_o[lo : lo + n_dev]
        xg = np.zeros((C, D), BF16)
        xg[:n_dev] = xf16[toks]
        xt = np.ascontiguousarray(xg.T.reshape(KO, P, C).transpose(1, 0, 2))
        in_maps.append(
            {"xt": xt, "w1": _pack_w1(gate_up[e]), "w2": _pack_w2(down[e])}
        )

    res = bass_utils.run_bass_kernel_spmd(
        nc, in_maps, core_ids=list(range(E)), trace=trace
    )

    contrib = np.empty((len(tok_o), D), np.float32)
    for e in range(E):
        lo, hi = offs[e], offs[e + 1]
        n_dev = dev_n[e]
        yt = np.asarray(res.results[e]["yt"]).astype(np.float32)  # [P, KO, C]
        ytd = yt.transpose(1, 0, 2).reshape(D, C)  # [D, C]
        contrib[lo : lo + n_dev] = ytd[:, :n_dev].T
        if n_dev < hi - lo:  # capacity overflow -> exact host FFN
            toks = tok_o[lo + n_dev : hi]
            contrib[lo + n_dev : hi] = _host_ffn(xf[toks], gate_up[e], down[e])

    contrib *= w_o[:, None]

    s = np.argsort(tok_o, kind="stable")
    tok_s = tok_o[s]
    out = np.zeros((N, D), np.float32)
    if len(tok_s) == 2 * N and np.array_equal(tok_s[0::2], tok_s[1::2]):
        cs = contrib[s]
        out[tok_s[0::2]] = cs[0::2] + cs[1::2]
    else:  # defensive fallback (duplicate experts per token can't happen)
        np.add.at(out, tok_o, contrib)
    return out.reshape(B, L, D), res


def kernel(x, router_scale, gate_w, gate_up, down, per_expert_scale):
    out, _ = run_moe(x, router_scale, gate_w, gate_up, down, per_expert_scale)
    return out


# revision 7
# speedup vs baseline: 1.0006x; 1.0006x over previous
"""MoE layer (top-2 of 8 experts) on 8 TRN2 NeuronCores, expert-parallel.

Host side: router (exact replica of the reference jax ops, so top-k
selection bit-matches), token gather by expert assignment, weight
repacking into DMA-friendly layouts, and the final weighted scatter-add.

Device side (one expert per core, SPMD): the full expert FFN
    h = X @ W1 ; act = gelu(h_gate) * h_up ; Y = act @ W2
computed in bf16 (fp32 PSUM accumulation, ~4e-3 rel err) with all
activations kept transposed (tokens on the free axis) so no on-device
transposes are needed.

Capacity-factor-1.0 expert parallelism: each core processes up to
C = N*TOP_K/n_cores tokens of its expert (two exact 512-column matmul
chunks); the <1% overflow tokens of over-subscribed experts are
computed exactly on host (fp32 BLAS) and merged in the combine step.

Self-contained: only library imports (numpy/jax/ml_dtypes/concourse),
no file reads.
"""

import numpy as np
import ml_dtypes

BF16 = ml_dtypes.bfloat16
TOP_K = 2
EPS = 1e-6
P = 128
D = 2048
F = 2048  # expert hidden dim (ED)
E = 8
KO = D // P  # 16 K-tiles for matmul1 / output D-tiles
MJ = F // P  # 16 gate/up tile pairs; also K-tiles for matmul2

_BUILD_CACHE: dict = {}

# Activation for the gate branch. CoreSim doesn't implement Gelu, so tests
# can set this to "Identity" for structural sim validation.
ACT_FN = "Gelu"


def _chunks_of(C: int) -> list[tuple[int, int]]:
    """Split the token-capacity free axis into matmul chunks <= 512."""
    if C <= 512:
        return [(0, C)]
    nch = -(-C // 512)
    base = C // nch
    base -= base % 8
    sizes = [base] * nch
    rem = C - base * nch
    i = 0
    while rem > 0:
        add = min(8, rem)
        sizes[i % nch] += add
        rem -= add
        i += 1
    out = []
    off = 0
    for s in sizes:
        out.append((off, s))
        off += s
    assert off == C
    return out


def _build(C: int):
    """Build + compile the per-core expert-FFN bass program for capacity C."""
    key = (C, ACT_FN)
    if key in _BUILD_CACHE:
        return _BUILD_CACHE[key]

    import concourse.bacc as bacc
    import concourse.mybir as mybir
    import concourse.tile as tile
    f32 = mybir.dt.float32
    bf16 = mybir.dt.bfloat16
    act_fn = getattr(mybir.ActivationFunctionType, ACT_FN)
    chunks = _chunks_of(C)

    nc = bacc.Bacc(
        "TRN2", target_bir_lowering=False, debug=False, enable_asserts=False
    )
    # Packed layouts (host pre-transposed, partition-major):
    #   xt[p, ko, c]    = X^T[ko*128+p, c]          (tokens on free axis)
    #   w1[p, m, ko, q] = W1perm[ko*128+p, m*128+q] (m: g0,u0,g1,u1,... strips)
    #   w2[p, i, fo, q] = W2[fo*128+p, i*128+q]
    #   yt[p, io, c]    = Y^T[io*128+p, c]
    xt_d = nc.dram_tensor("xt", [P, KO, C], bf16, kind="ExternalInput")
    w1_d = nc.dram_tensor("w1", [P, 2 * MJ, KO, P], bf16, kind="ExternalInput")
    w2_d = nc.dram_tensor("w2", [P, KO, MJ, P], bf16, kind="ExternalInput")
    yt_d = nc.dram_tensor("yt", [P, KO, C], bf16, kind="ExternalOutput")

    NWARM = 30  # PE warm-up matmuls covering the DMA-latency head (~11us)

    with tile.TileContext(nc) as tc:
        with (
            tc.tile_pool(name="xt", bufs=1) as xt_pool,
            tc.tile_pool(name="act", bufs=1) as act_pool,
            tc.tile_pool(name="wu", bufs=1) as wu_pool,
            tc.tile_pool(name="w1", bufs=8) as w1_pool,
            tc.tile_pool(name="w2", bufs=2) as w2_pool,
            tc.tile_pool(name="tg", bufs=3) as tg_pool,
            tc.tile_pool(name="yo", bufs=3) as yo_pool,
            tc.tile_pool(name="ps", bufs=8, space="PSUM") as ps_pool,
        ):
            nch = len(chunks)

            # PE warm-up: dummy matmuls on memset data, issued ahead of the
            # real stream. They run while the head DMAs are in flight, so the
            # PE is at full clock (and HAM at K=8/8) when real data lands,
            # instead of paying the 1.2 GHz ramp + idle-triggered throttle.
            wz = wu_pool.tile([P, P], bf16)
            wx = wu_pool.tile([P, 512], bf16)
            nc.vector.memset(wz[:], 0.0)
            nc.vector.memset(wx[:], 0.0)
            pw = ps_pool.tile([P, 512], f32, tag="ps")
            for _ in range(NWARM):
                nc.tensor.matmul(pw[:], wz[:], wx[:], start=True, stop=True)

            # Head DMA plan, in PE demand order, x spread across both HWDGE
            # rings. The ffn1 unit order below keeps the first three j's on
            # chunk 0 only, so chunk 1 of x isn't needed until ~33us in.
            w1_tiles = {}

            def issue_w1(m, ring):
                t = w1_pool.tile([P, KO, P], bf16, tag="w1s")
                ring.dma_start(t[:], w1_d.ap()[:, m])
                w1_tiles[m] = t

            xt_sb = xt_pool.tile([P, KO, C], bf16)

            def issue_xt(ring, ci, k0, k1):
                c0, cn = chunks[ci]
                ring.dma_start(
                    xt_sb[:, k0:k1, c0 : c0 + cn], xt_d.ap()[:, k0:k1, c0 : c0 + cn]
                )

            t = w1_pool.tile([P, KO, P], bf16, tag="w1s")
            nc.sync.dma_start(t[:, : KO // 2], w1_d.ap()[:, 0, : KO // 2])
            issue_xt(nc.sync, 0, 0, 4)
            issue_xt(nc.scalar, 0, 4, 8)
            nc.sync.dma_start(t[:, KO // 2 :], w1_d.ap()[:, 0, KO // 2 :])
            w1_tiles[0] = t
            issue_xt(nc.scalar, 0, 8, 12)
            t = w1_pool.tile([P, KO, P], bf16, tag="w1s")
            nc.sync.dma_start(t[:, : KO // 2], w1_d.ap()[:, 1, : KO // 2])
            issue_xt(nc.scalar, 0, 12, 16)
            nc.sync.dma_start(t[:, KO // 2 :], w1_d.ap()[:, 1, KO // 2 :])
            w1_tiles[1] = t
            issue_w1(2, nc.sync)
            issue_w1(3, nc.scalar)
            issue_w1(4, nc.scalar)
            issue_w1(5, nc.scalar)
            if nch >= 2:
                issue_xt(nc.sync, 1, 0, 4)
                issue_xt(nc.sync, 1, 4, 8)
                issue_xt(nc.scalar, 1, 8, 12)
                issue_xt(nc.scalar, 1, 12, 16)
            for ci in range(2, nch):
                for k0 in range(0, KO, 4):
                    issue_xt(nc.scalar, ci, k0, k0 + 4)
            act_sb = act_pool.tile([P, MJ, C], bf16)

            # ffn1 unit order: chunk-major over the first three (preloaded)
            # j's so the early PE demand matches the head DMA supply, then
            # j-major for the rest.
            head_js = min(3, MJ)
            order = [(j, ci) for ci in range(nch) for j in range(head_js)]
            order += [(j, ci) for j in range(head_js, MJ) for ci in range(nch)]
            last_unit = {}
            for idx, (j, ci) in enumerate(order):
                last_unit[j] = idx

            def get_w1(m):
                if m not in w1_tiles:
                    t = w1_pool.tile([P, KO, P], bf16, tag="w1s")
                    nc.scalar.dma_start(t[:], w1_d.ap()[:, m])
                    w1_tiles[m] = t
                return w1_tiles[m]

            with nc.named_scope("ffn1"):
                for idx, (j, ci) in enumerate(order):
                    wg = get_w1(2 * j)
                    wu = get_w1(2 * j + 1)
                    c0, cn = chunks[ci]
                    pg = ps_pool.tile([P, 512], f32, tag="ps")
                    pu = ps_pool.tile([P, 512], f32, tag="ps")
                    for ko in range(KO):
                        nc.tensor.matmul(
                            pg[:, :cn],
                            wg[:, ko],
                            xt_sb[:, ko, c0 : c0 + cn],
                            start=(ko == 0),
                            stop=(ko == KO - 1),
                        )
                    for ko in range(KO):
                        nc.tensor.matmul(
                            pu[:, :cn],
                            wu[:, ko],
                            xt_sb[:, ko, c0 : c0 + cn],
                            start=(ko == 0),
                            stop=(ko == KO - 1),
                        )
                    tg = tg_pool.tile([P, 512], f32, tag="tg")
                    nc.scalar.activation(tg[:, :cn], pg[:, :cn], act_fn)
                    nc.vector.tensor_mul(
                        out=act_sb[:, j, c0 : c0 + cn],
                        in0=tg[:, :cn],
                        in1=pu[:, :cn],
                    )
                    if idx == last_unit[j]:
                        w1_tiles.pop(2 * j, None)
                        w1_tiles.pop(2 * j + 1, None)

            with nc.named_scope("ffn2"):
                for i in range(KO):
                    w2t = w2_pool.tile([P, MJ, P], bf16, tag="w2s")
                    (nc.sync if i < 2 else nc.scalar).dma_start(
                        w2t[:], w2_d.ap()[:, i]
                    )
                    units = list(chunks)
                    if i == KO - 1 and units[-1][1] >= 256:
                        c0, cn = units.pop()
                        h = (cn // 2 + 7) // 8 * 8
                        q = (cn - h) // 2 // 8 * 8
                        units += [(c0, h), (c0 + h, cn - h - q), (c0 + cn - q, q)]
                    for c0, cn in units:
                        py = ps_pool.tile([P, 512], f32, tag="ps")
                        for fo in range(MJ):
                            nc.tensor.matmul(
                                py[:, :cn],
                                w2t[:, fo],
                                act_sb[:, fo, c0 : c0 + cn],
                                start=(fo == 0),
                                stop=(fo == MJ - 1),
                            )
                        yo = yo_pool.tile([P, 512], bf16, tag="yo")
                        nc.vector.tensor_copy(out=yo[:, :cn], in_=py[:, :cn])
                        nc.sync.dma_start(yt_d.ap()[:, i, c0 : c0 + cn], yo[:, :cn])

    nc.compile()
    _BUILD_CACHE[key] = nc
    return nc


def _router(x, router_scale, gate_w):
    """Replicate the reference router ops exactly (same jax ops, default
    backend) so the top-2 expert selection bit-matches the reference."""
    import jax
    import jax.numpy as jnp

    x = jnp.asarray(x)
    router_scale = jnp.asarray(router_scale)
    gate_w = jnp.asarray(gate_w)
    _B, _L, d = x.shape
    h = x * jax.lax.rsqrt(jnp.mean(x * x, axis=-1, keepdims=True) + EPS)
    h = h * (d**-0.5) * router_scale
    logits = (h @ gate_w).astype(jnp.float32)
    probs = jax.nn.softmax(logits, axis=-1)
    w, idx = jax.lax.top_k(probs, TOP_K)
    w = w / jnp.clip(jnp.sum(w, axis=-1, keepdims=True), 1e-12)
    w = w.astype(x.dtype)
    return (
        np.asarray(idx).reshape(-1, TOP_K),
        np.asarray(w).reshape(-1, TOP_K).astype(np.float32),
    )


def _gelu_exact(v: np.ndarray) -> np.ndarray:
    """gelu(approximate=False) = v * Phi(v), matching jax.nn.gelu."""
    try:
        from scipy.special import erf
    except Exception:  # pragma: no cover - scipy ships with jax
        import math

        erf = np.vectorize(math.erf, otypes=[np.float64])
    return (v * 0.5 * (1.0 + erf(v / np.sqrt(2.0)))).astype(np.float32)


def _host_ffn(xe: np.ndarray, gate_up_e: np.ndarray, down_e: np.ndarray):
    """Exact fp32 expert FFN for the few capacity-overflow tokens."""
    h = xe @ gate_up_e
    act = _gelu_exact(h[:, :F]) * h[:, F:]
    return act @ down_e


def _pack_w1(gate_up_e: np.ndarray) -> np.ndarray:
    """[D, 2F] -> [P, 2*MJ, KO, P] with gate/up 128-col strips interleaved."""
    g = gate_up_e[:, :F].reshape(D, MJ, P)
    u = gate_up_e[:, F:].reshape(D, MJ, P)
    w1p = np.empty((D, 2 * MJ, P), np.float32)
    w1p[:, 0::2] = g
    w1p[:, 1::2] = u
    # [D, 2MJ, P] -> [KO, P, 2MJ, P] -> [P, 2MJ, KO, P]
    return np.ascontiguousarray(
        w1p.reshape(KO, P, 2 * MJ, P).transpose(1, 2, 0, 3).astype(BF16)
    )


def _pack_w2(down_e: np.ndarray) -> np.ndarray:
    """[F, D] -> [P, KO, MJ, P]  (w2[p, i, fo, q] = W2[fo*128+p, i*128+q])."""
    return np.ascontiguousarray(
        down_e.reshape(MJ, P, KO, P).transpose(1, 2, 0, 3).astype(BF16)
    )


def run_moe(x, router_scale, gate_w, gate_up, down, per_expert_scale, trace=False):
    from concourse import bass_utils

    x = np.asarray(x, dtype=np.float32)
    router_scale = np.asarray(router_scale, dtype=np.float32)
    gate_w = np.asarray(gate_w, dtype=np.float32)
    gate_up = np.asarray(gate_up, dtype=np.float32)
    down = np.asarray(down, dtype=np.float32)
    per_expert_scale = np.asarray(per_expert_scale, dtype=np.float32)

    B, L, d = x.shape
    N = B * L
    assert d == D and gate_up.shape == (E, D, 2 * F) and down.shape == (E, F, D)

    idxf, wf = _router(x, router_scale, gate_w)

    pair_expert = idxf.reshape(-1)
    pair_token = np.repeat(np.arange(N), TOP_K)
    pair_w = wf.reshape(-1) * per_expert_scale[pair_expert]

    order = np.argsort(pair_expert, kind="stable")
    tok_o = pair_token[order]
    w_o = pair_w[order]
    counts = np.bincount(pair_expert, minlength=E)
    offs = np.zeros(E + 1, np.int64)
    offs[1:] = np.cumsum(counts)

    # Capacity factor 1.0: each core takes up to C tokens of its expert;
    # the <1% overflow of over-subscribed experts is computed exactly on
    # host and merged below.
    CMAX = max(512, -(-(N * TOP_K // E) // 8) * 8)
    C = max(64, min(CMAX, -(-int(counts.max()) // 8) * 8))

    nc = _build(C)

    xf = x.reshape(N, D)
    xf16 = xf.astype(BF16)

    in_maps = []
    dev_n = []
    for e in range(E):
        lo, hi = offs[e], offs[e + 1]
        n_dev = min(C, hi - lo)
        dev_n.append(n_dev)
        toks = tok_o[lo : lo + n_dev]
        xg = np.zeros((C, D), BF16)
        xg[:n_dev] = xf16[toks]
        xt = np.ascontiguousarray(xg.T.reshape(KO, P, C).transpose(1, 0, 2))
        in_maps.append(
            {"xt": xt, "w1": _pack_w1(gate_up[e]), "w2": _pack_w2(down[e])}
        )

    res = bass_utils.run_bass_kernel_spmd(
        nc, in_maps, core_ids=list(range(E)), trace=trace
    )

    contrib = np.empty((len(tok_o), D), np.float32)
    for e in range(E):
        lo, hi = offs[e], offs[e + 1]
        n_dev = dev_n[e]
        yt = np.asarray(res.results[e]["yt"]).astype(np.float32)  # [P, KO, C]
        ytd = yt.transpose(1, 0, 2).reshape(D, C)  # [D, C]
        contrib[lo : lo + n_dev] = ytd[:, :n_dev].T
        if n_dev < hi - lo:  # capacity overflow -> exact host FFN
            toks = tok_o[lo + n_dev : hi]
            contrib[lo + n_dev : hi] = _host_ffn(xf[toks], gate_up[e], down[e])

    contrib *= w_o[:, None]

    s = np.argsort(tok_o, kind="stable")
    tok_s = tok_o[s]
    out = np.zeros((N, D), np.float32)
    if len(tok_s) == 2 * N and np.array_equal(tok_s[0::2], tok_s[1::2]):
        cs = contrib[s]
        out[tok_s[0::2]] = cs[0::2] + cs[1::2]
    else:  # defensive fallback (duplicate experts per token can't happen)
        np.add.at(out, tok_o, contrib)
    return out.reshape(B, L, D), res


def kernel(x, router_scale, gate_w, gate_up, down, per_expert_scale):
    out, _ = run_moe(x, router_scale, gate_w, gate_up, down, per_expert_scale)
    return out


# revision 15
# speedup vs baseline: 1.0118x; 1.0111x over previous
"""MoE layer (top-2 of 8 experts) on 8 TRN2 NeuronCores, expert-parallel.

Host side: router (exact replica of the reference jax ops, so top-k
selection bit-matches), token gather by expert assignment, weight
repacking into DMA-friendly layouts, and the final weighted scatter-add.

Device side (one expert per core, SPMD): the full expert FFN
    h = X @ W1 ; act = gelu(h_gate) * h_up ; Y = act @ W2
computed in bf16 (fp32 PSUM accumulation, ~4e-3 rel err) with all
activations kept transposed (tokens on the free axis) so no on-device
transposes are needed.

Capacity-factor-1.0 expert parallelism: each core processes up to
C = N*TOP_K/n_cores tokens of its expert (two exact 512-column matmul
chunks); the <1% overflow tokens of over-subscribed experts are
computed exactly on host (fp32 BLAS) and merged in the combine step.

Self-contained: only library imports (numpy/jax/ml_dtypes/concourse),
no file reads.
"""

import numpy as np
import ml_dtypes

BF16 = ml_dtypes.bfloat16
FP8 = ml_dtypes.float8_e4m3
DR_J0 = 3  # first gate tile-pair index using the fp8 DoubleRow tail
TOP_K = 2
EPS = 1e-6
P = 128
D = 2048
F = 2048  # expert hidden dim (ED)
E = 8
KO = D // P  # 16 K-tiles for matmul1 / output D-tiles
MJ = F // P  # 16 gate/up tile pairs; also K-tiles for matmul2

_BUILD_CACHE: dict = {}

# Activation for the gate branch. CoreSim doesn't implement Gelu, so tests
# can set this to "Identity" for structural sim validation.
ACT_FN = "Gelu"


def _chunks_of(C: int) -> list[tuple[int, int]]:
    """Split the token-capacity free axis into matmul chunks <= 512."""
    if C <= 512:
        return [(0, C)]
    nch = -(-C // 512)
    base = C // nch
    base -= base % 8
    sizes = [base] * nch
    rem = C - base * nch
    i = 0
    while rem > 0:
        add = min(8, rem)
        sizes[i % nch] += add
        rem -= add
        i += 1
    out = []
    off = 0
    for s in sizes:
        out.append((off, s))
        off += s
    assert off == C
    return out


def _build(C: int):
    """Build + compile the per-core expert-FFN bass program for capacity C."""
    key = (C, ACT_FN)
    if key in _BUILD_CACHE:
        return _BUILD_CACHE[key]

    import concourse.bacc as bacc
    import concourse.mybir as mybir
    import concourse.tile as tile
    f32 = mybir.dt.float32
    bf16 = mybir.dt.bfloat16
    fp8 = mybir.dt.float8e4
    DR = mybir.MatmulPerfMode.DoubleRow
    act_fn = getattr(mybir.ActivationFunctionType, ACT_FN)
    chunks = _chunks_of(C)

    nc = bacc.Bacc(
        "TRN2", target_bir_lowering=False, debug=False, enable_asserts=False
    )
    # Packed layouts (host pre-transposed, partition-major):
    #   xt[p, ko, c]    = X^T[ko*128+p, c]          (tokens on free axis)
    #   w1[p, m, ko, q] = W1perm[ko*128+p, m*128+q] (m: g0,u0,g1,u1,... strips)
    #   w2[p, i, fo, q] = W2[fo*128+p, i*128+q]
    #   yt[p, io, c]    = Y^T[io*128+p, c]
    xt_d = nc.dram_tensor("xt", [P, KO, C], bf16, kind="ExternalInput")
    w1_d = nc.dram_tensor("w1", [P, 2 * MJ, KO, P], bf16, kind="ExternalInput")
    w2_d = nc.dram_tensor("w2", [P, KO, MJ, P], bf16, kind="ExternalInput")
    yt_d = nc.dram_tensor("yt", [P, KO, C], bf16, kind="ExternalOutput")
    # fp8 pair-packed tails (last two K-tiles) of x and the gate strips, for
    # the DoubleRow matmul that replaces the last two bf16 K-tiles of the
    # gate branch on j >= DR_J0 (error budget: ~1.3e-2 vs the 2e-2 gate).
    x8_d = nc.dram_tensor("x8", [P, 2, C], fp8, kind="ExternalInput")
    w8_d = nc.dram_tensor("w8", [P, 2, MJ, P], fp8, kind="ExternalInput")

    NWARM = 30  # PE warm-up matmuls covering the DMA-latency head (~11us)

    with tile.TileContext(nc) as tc:
        with (
            tc.tile_pool(name="xt", bufs=1) as xt_pool,
            tc.tile_pool(name="act", bufs=1) as act_pool,
            tc.tile_pool(name="wu", bufs=1) as wu_pool,
            tc.tile_pool(name="x8", bufs=1) as x8_pool,
            tc.tile_pool(name="w8", bufs=4) as w8_pool,
            tc.tile_pool(name="w1", bufs=8) as w1_pool,
            tc.tile_pool(name="w2", bufs=2) as w2_pool,
            tc.tile_pool(name="tg", bufs=3) as tg_pool,
            tc.tile_pool(name="yo", bufs=3) as yo_pool,
            tc.tile_pool(name="ps", bufs=8, space="PSUM") as ps_pool,
        ):
            nch = len(chunks)

            # PE warm-up: dummy matmuls on memset data, issued ahead of the
            # real stream. They run while the head DMAs are in flight, so the
            # PE is at full clock (and HAM at K=8/8) when real data lands,
            # instead of paying the 1.2 GHz ramp + idle-triggered throttle.
            wz = wu_pool.tile([P, P], bf16)
            wx = wu_pool.tile([P, 512], bf16)
            nc.vector.memset(wz[:], 0.0)
            nc.vector.memset(wx[:], 0.0)
            pw = ps_pool.tile([P, 512], f32, tag="ps")
            for _ in range(NWARM):
                nc.tensor.matmul(pw[:], wz[:], wx[:], start=True, stop=True)

            # Head DMA plan, in PE demand order, x spread across both HWDGE
            # rings. The ffn1 unit order below keeps the first three j's on
            # chunk 0 only, so chunk 1 of x isn't needed until ~33us in.
            w1_tiles = {}

            def issue_w1(m, ring):
                t = w1_pool.tile([P, KO, P], bf16, tag="w1s")
                ring.dma_start(t[:], w1_d.ap()[:, m])
                w1_tiles[m] = t

            xt_sb = xt_pool.tile([P, KO, C], bf16)

            def issue_xt(ring, ci, k0, k1):
                c0, cn = chunks[ci]
                ring.dma_start(
                    xt_sb[:, k0:k1, c0 : c0 + cn], xt_d.ap()[:, k0:k1, c0 : c0 + cn]
                )

            t = w1_pool.tile([P, KO, P], bf16, tag="w1s")
            nc.sync.dma_start(t[:, : KO // 2], w1_d.ap()[:, 0, : KO // 2])
            issue_xt(nc.sync, 0, 0, 4)
            issue_xt(nc.scalar, 0, 4, 8)
            nc.sync.dma_start(t[:, KO // 2 :], w1_d.ap()[:, 0, KO // 2 :])
            w1_tiles[0] = t
            issue_xt(nc.scalar, 0, 8, 12)
            t = w1_pool.tile([P, KO, P], bf16, tag="w1s")
            nc.sync.dma_start(t[:, : KO // 2], w1_d.ap()[:, 1, : KO // 2])
            issue_xt(nc.scalar, 0, 12, 16)
            nc.sync.dma_start(t[:, KO // 2 :], w1_d.ap()[:, 1, KO // 2 :])
            w1_tiles[1] = t
            issue_w1(2, nc.sync)
            issue_w1(3, nc.scalar)
            issue_w1(4, nc.scalar)
            issue_w1(5, nc.scalar)
            if nch >= 2:
                issue_xt(nc.sync, 1, 0, 4)
                issue_xt(nc.sync, 1, 4, 8)
                issue_xt(nc.scalar, 1, 8, 12)
                issue_xt(nc.scalar, 1, 12, 16)
            for ci in range(2, nch):
                for k0 in range(0, KO, 4):
                    issue_xt(nc.scalar, ci, k0, k0 + 4)
            x8_sb = x8_pool.tile([P, 2, C], fp8)
            nc.scalar.dma_start(x8_sb[:], x8_d.ap()[:])
            w8_tiles = {}

            def get_w8(j):
                if j not in w8_tiles:
                    t = w8_pool.tile([P, 2, P], fp8, tag="w8s")
                    nc.scalar.dma_start(t[:], w8_d.ap()[:, :, j])
                    w8_tiles[j] = t
                return w8_tiles[j]

            act_sb = act_pool.tile([P, MJ, C], bf16)

            # ffn1 unit order: chunk-major over the first three (preloaded)
            # j's so the early PE demand matches the head DMA supply, then
            # j-major for the rest.
            head_js = min(3, MJ)
            order = [(j, ci) for ci in range(nch) for j in range(head_js)]
            order += [(j, ci) for j in range(head_js, MJ) for ci in range(nch)]
            last_unit = {}
            for idx, (j, ci) in enumerate(order):
                last_unit[j] = idx

            def get_w1(m):
                if m not in w1_tiles:
                    t = w1_pool.tile([P, KO, P], bf16, tag="w1s")
                    nc.scalar.dma_start(t[:], w1_d.ap()[:, m])
                    w1_tiles[m] = t
                return w1_tiles[m]

            with nc.named_scope("ffn1"):
                for idx, (j, ci) in enumerate(order):
                    wg = get_w1(2 * j)
                    wu = get_w1(2 * j + 1)
                    c0, cn = chunks[ci]
                    dr = j >= DR_J0
                    pg = ps_pool.tile([P, 512], f32, tag="ps")
                    pu = ps_pool.tile([P, 512], f32, tag="ps")
                    nko_g = KO - 2 if dr else KO
                    for ko in range(nko_g):
                        nc.tensor.matmul(
                            pg[:, :cn],
                            wg[:, ko],
                            xt_sb[:, ko, c0 : c0 + cn],
                            start=(ko == 0),
                            stop=(ko == nko_g - 1) and not dr,
                            skip_group_check=dr,
                        )
                    if dr:
                        nc.tensor.matmul(
                            pg[:, :cn],
                            get_w8(j)[:],
                            x8_sb[:, :, c0 : c0 + cn],
                            start=False,
                            stop=True,
                            perf_mode=DR,
                            skip_group_check=True,
                        )
                    for ko in range(KO):
                        nc.tensor.matmul(
                            pu[:, :cn],
                            wu[:, ko],
                            xt_sb[:, ko, c0 : c0 + cn],
                            start=(ko == 0),
                            stop=(ko == KO - 1),
                        )
                    tg = tg_pool.tile([P, 512], f32, tag="tg")
                    nc.scalar.activation(tg[:, :cn], pg[:, :cn], act_fn)
                    nc.vector.tensor_mul(
                        out=act_sb[:, j, c0 : c0 + cn],
                        in0=tg[:, :cn],
                        in1=pu[:, :cn],
                    )
                    if idx == last_unit[j]:
                        w1_tiles.pop(2 * j, None)
                        w1_tiles.pop(2 * j + 1, None)
                        w8_tiles.pop(j, None)

            with nc.named_scope("ffn2"):
                for i in range(KO):
                    w2t = w2_pool.tile([P, MJ, P], bf16, tag="w2s")
                    (nc.sync if i < 2 else nc.scalar).dma_start(
                        w2t[:], w2_d.ap()[:, i]
                    )
                    units = list(chunks)
                    if i == KO - 1 and units[-1][1] >= 256:
                        c0, cn = units.pop()
                        h = (cn // 2 + 7) // 8 * 8
                        q = (cn - h) // 2 // 8 * 8
                        units += [(c0, h), (c0 + h, cn - h - q), (c0 + cn - q, q)]
                    for c0, cn in units:
                        py = ps_pool.tile([P, 512], f32, tag="ps")
                        for fo in range(MJ):
                            nc.tensor.matmul(
                                py[:, :cn],
                                w2t[:, fo],
                                act_sb[:, fo, c0 : c0 + cn],
                                start=(fo == 0),
                                stop=(fo == MJ - 1),
                            )
                        yo = yo_pool.tile([P, 512], bf16, tag="yo")
                        nc.vector.tensor_copy(out=yo[:, :cn], in_=py[:, :cn])
                        nc.sync.dma_start(yt_d.ap()[:, i, c0 : c0 + cn], yo[:, :cn])

    nc.compile()
    _BUILD_CACHE[key] = nc
    return nc


def _router(x, router_scale, gate_w):
    """Replicate the reference router ops exactly (same jax ops, default
    backend) so the top-2 expert selection bit-matches the reference."""
    import jax
    import jax.numpy as jnp

    x = jnp.asarray(x)
    router_scale = jnp.asarray(router_scale)
    gate_w = jnp.asarray(gate_w)
    _B, _L, d = x.shape
    h = x * jax.lax.rsqrt(jnp.mean(x * x, axis=-1, keepdims=True) + EPS)
    h = h * (d**-0.5) * router_scale
    logits = (h @ gate_w).astype(jnp.float32)
    probs = jax.nn.softmax(logits, axis=-1)
    w, idx = jax.lax.top_k(probs, TOP_K)
    w = w / jnp.clip(jnp.sum(w, axis=-1, keepdims=True), 1e-12)
    w = w.astype(x.dtype)
    return (
        np.asarray(idx).reshape(-1, TOP_K),
        np.asarray(w).reshape(-1, TOP_K).astype(np.float32),
    )


def _gelu_exact(v: np.ndarray) -> np.ndarray:
    """gelu(approximate=False) = v * Phi(v), matching jax.nn.gelu."""
    try:
        from scipy.special import erf
    except Exception:  # pragma: no cover - scipy ships with jax
        import math

        erf = np.vectorize(math.erf, otypes=[np.float64])
    return (v * 0.5 * (1.0 + erf(v / np.sqrt(2.0)))).astype(np.float32)


def _host_ffn(xe: np.ndarray, gate_up_e: np.ndarray, down_e: np.ndarray):
    """Exact fp32 expert FFN for the few capacity-overflow tokens."""
    h = xe @ gate_up_e
    act = _gelu_exact(h[:, :F]) * h[:, F:]
    return act @ down_e


def _pack_w1(gate_up_e: np.ndarray) -> np.ndarray:
    """[D, 2F] -> [P, 2*MJ, KO, P] with gate/up 128-col strips interleaved."""
    g = gate_up_e[:, :F].reshape(D, MJ, P)
    u = gate_up_e[:, F:].reshape(D, MJ, P)
    w1p = np.empty((D, 2 * MJ, P), np.float32)
    w1p[:, 0::2] = g
    w1p[:, 1::2] = u
    # [D, 2MJ, P] -> [KO, P, 2MJ, P] -> [P, 2MJ, KO, P]
    return np.ascontiguousarray(
        w1p.reshape(KO, P, 2 * MJ, P).transpose(1, 2, 0, 3).astype(BF16)
    )


def _pack_w2(down_e: np.ndarray) -> np.ndarray:
    """[F, D] -> [P, KO, MJ, P]  (w2[p, i, fo, q] = W2[fo*128+p, i*128+q])."""
    return np.ascontiguousarray(
        down_e.reshape(MJ, P, KO, P).transpose(1, 2, 0, 3).astype(BF16)
    )


def run_moe(x, router_scale, gate_w, gate_up, down, per_expert_scale, trace=False):
    from concourse import bass_utils

    x = np.asarray(x, dtype=np.float32)
    router_scale = np.asarray(router_scale, dtype=np.float32)
    gate_w = np.asarray(gate_w, dtype=np.float32)
    gate_up = np.asarray(gate_up, dtype=np.float32)
    down = np.asarray(down, dtype=np.float32)
    per_expert_scale = np.asarray(per_expert_scale, dtype=np.float32)

    B, L, d = x.shape
    N = B * L
    assert d == D and gate_up.shape == (E, D, 2 * F) and down.shape == (E, F, D)

    idxf, wf = _router(x, router_scale, gate_w)

    pair_expert = idxf.reshape(-1)
    pair_token = np.repeat(np.arange(N), TOP_K)
    pair_w = wf.reshape(-1) * per_expert_scale[pair_expert]

    order = np.argsort(pair_expert, kind="stable")
    tok_o = pair_token[order]
    w_o = pair_w[order]
    counts = np.bincount(pair_expert, minlength=E)
    offs = np.zeros(E + 1, np.int64)
    offs[1:] = np.cumsum(counts)

    # Capacity factor 1.0: each core takes up to C tokens of its expert;
    # the <1% overflow of over-subscribed experts is computed exactly on
    # host and merged below.
    CMAX = max(512, -(-(N * TOP_K // E) // 8) * 8)
    C = max(64, min(CMAX, -(-int(counts.max()) // 8) * 8))

    nc = _build(C)

    xf = x.reshape(N, D)
    xf16 = xf.astype(BF16)

    in_maps = []
    dev_n = []
    for e in range(E):
        lo, hi = offs[e], offs[e + 1]
        n_dev = min(C, hi - lo)
        dev_n.append(n_dev)
        toks = tok_o[lo : lo + n_dev]
        xg = np.zeros((C, D), BF16)
        xg[:n_dev] = xf16[toks]
        xt = np.ascontiguousarray(xg.T.reshape(KO, P, C).transpose(1, 0, 2))
        # fp8 pair-packed tails: x8[p,i,c] = X[c, (KO-2+i)*P+p],
        # w8[p,i,j,q] = gate_up[e][(KO-2+i)*P+p, j*P+q] (gate strips only).
        xg8 = np.zeros((C, 2 * P), np.float32)
        xg8[:n_dev] = xf[toks][:, (KO - 2) * P :]
        x8 = np.ascontiguousarray(
            xg8.T.reshape(2, P, C).transpose(1, 0, 2).astype(FP8)
        )
        w8 = np.ascontiguousarray(
            gate_up[e][(KO - 2) * P :, :F]
            .reshape(2, P, MJ, P)
            .transpose(1, 0, 2, 3)
            .astype(FP8)
        )
        in_maps.append(
            {
                "xt": xt,
                "w1": _pack_w1(gate_up[e]),
                "w2": _pack_w2(down[e]),
                "x8": x8,
                "w8": w8,
            }
        )

    res = bass_utils.run_bass_kernel_spmd(
        nc, in_maps, core_ids=list(range(E)), trace=trace
    )

    contrib = np.empty((len(tok_o), D), np.float32)
    for e in range(E):
        lo, hi = offs[e], offs[e + 1]
        n_dev = dev_n[e]
        yt = np.asarray(res.results[e]["yt"]).astype(np.float32)  # [P, KO, C]
        ytd = yt.transpose(1, 0, 2).reshape(D, C)  # [D, C]
        contrib[lo : lo + n_dev] = ytd[:, :n_dev].T
        if n_dev < hi - lo:  # capacity overflow -> exact host FFN
            toks = tok_o[lo + n_dev : hi]
            contrib[lo + n_dev : hi] = _host_ffn(xf[toks], gate_up[e], down[e])

    contrib *= w_o[:, None]

    s = np.argsort(tok_o, kind="stable")
    tok_s = tok_o[s]
    out = np.zeros((N, D), np.float32)
    if len(tok_s) == 2 * N and np.array_equal(tok_s[0::2], tok_s[1::2]):
        cs = contrib[s]
        out[tok_s[0::2]] = cs[0::2] + cs[1::2]
    else:  # defensive fallback (duplicate experts per token can't happen)
        np.add.at(out, tok_o, contrib)
    return out.reshape(B, L, D), res


def kernel(x, router_scale, gate_w, gate_up, down, per_expert_scale):
    out, _ = run_moe(x, router_scale, gate_w, gate_up, down, per_expert_scale)
    return out


# revision 20
# speedup vs baseline: 1.0296x; 1.0176x over previous
"""MoE layer (top-2 of 8 experts) on 8 TRN2 NeuronCores, expert-parallel.

Host side: router (exact replica of the reference jax ops, so top-k
selection bit-matches), token gather by expert assignment, weight
repacking into DMA-friendly layouts, and the final weighted scatter-add.

Device side (one expert per core, SPMD): the full expert FFN
    h = X @ W1 ; act = gelu(h_gate) * h_up ; Y = act @ W2
computed in bf16 (fp32 PSUM accumulation, ~4e-3 rel err) with all
activations kept transposed (tokens on the free axis) so no on-device
transposes are needed.

Capacity-factor-1.0 expert parallelism: each core processes up to
C = N*TOP_K/n_cores tokens of its expert (two exact 512-column matmul
chunks); the <1% overflow tokens of over-subscribed experts are
computed exactly on host (fp32 BLAS) and merged in the combine step.

Self-contained: only library imports (numpy/jax/ml_dtypes/concourse),
no file reads.
"""

import numpy as np
import ml_dtypes

BF16 = ml_dtypes.bfloat16
FP8 = ml_dtypes.float8_e4m3
DR_J0 = 3  # first gate tile-pair index using the fp8 DoubleRow tail
TOP_K = 2
EPS = 1e-6
P = 128
D = 2048
F = 2048  # expert hidden dim (ED)
E = 8
KO = D // P  # 16 K-tiles for matmul1 / output D-tiles
MJ = F // P  # 16 gate/up tile pairs; also K-tiles for matmul2

_BUILD_CACHE: dict = {}

# Activation for the gate branch. CoreSim doesn't implement Gelu, so tests
# can set this to "Identity" for structural sim validation.
ACT_FN = "Gelu"


def _chunks_of(C: int) -> list[tuple[int, int]]:
    """Split the token-capacity free axis into matmul chunks <= 512."""
    if C <= 512:
        return [(0, C)]
    nch = -(-C // 512)
    base = C // nch
    base -= base % 8
    sizes = [base] * nch
    rem = C - base * nch
    i = 0
    while rem > 0:
        add = min(8, rem)
        sizes[i % nch] += add
        rem -= add
        i += 1
    out = []
    off = 0
    for s in sizes:
        out.append((off, s))
        off += s
    assert off == C
    return out


def _build(C: int):
    """Build + compile the per-core expert-FFN bass program for capacity C."""
    key = (C, ACT_FN)
    if key in _BUILD_CACHE:
        return _BUILD_CACHE[key]

    import concourse.bacc as bacc
    import concourse.mybir as mybir
    import concourse.tile as tile
    f32 = mybir.dt.float32
    bf16 = mybir.dt.bfloat16
    fp8 = mybir.dt.float8e4
    DR = mybir.MatmulPerfMode.DoubleRow
    act_fn = getattr(mybir.ActivationFunctionType, ACT_FN)
    chunks = _chunks_of(C)

    nc = bacc.Bacc(
        "TRN2", target_bir_lowering=False, debug=False, enable_asserts=False
    )
    # Packed layouts (host pre-transposed, partition-major):
    #   xt[p, ko, c]    = X^T[ko*128+p, c]          (tokens on free axis)
    #   w1[p, m, ko, q] = W1perm[ko*128+p, m*128+q] (m: g0,u0,g1,u1,... strips)
    #   w2[p, i, fo, q] = W2[fo*128+p, i*128+q]
    #   yt[p, io, c]    = Y^T[io*128+p, c]
    xt_d = nc.dram_tensor("xt", [P, KO, C], bf16, kind="ExternalInput")
    w1_d = nc.dram_tensor("w1", [P, 2 * MJ, KO, P], bf16, kind="ExternalInput")
    w2_d = nc.dram_tensor("w2", [P, KO, MJ, P], bf16, kind="ExternalInput")
    yt_d = nc.dram_tensor("yt", [P, KO, C], bf16, kind="ExternalOutput")
    # fp8 pair-packed tails (last two K-tiles) of x and the gate strips, for
    # the DoubleRow matmul that replaces the last two bf16 K-tiles of the
    # gate branch on j >= DR_J0 (error budget: ~1.3e-2 vs the 2e-2 gate).
    x8_d = nc.dram_tensor("x8", [P, 2, C], fp8, kind="ExternalInput")
    w8_d = nc.dram_tensor("w8", [P, 2, MJ, P], fp8, kind="ExternalInput")
    w8u_d = nc.dram_tensor("w8u", [P, 2, MJ, P], fp8, kind="ExternalInput")

    NWARM = 30  # PE warm-up matmuls covering the DMA-latency head (~11us)

    with tile.TileContext(nc) as tc:
        with (
            tc.tile_pool(name="xt", bufs=1) as xt_pool,
            tc.tile_pool(name="act", bufs=1) as act_pool,
            tc.tile_pool(name="wu", bufs=1) as wu_pool,
            tc.tile_pool(name="x8", bufs=1) as x8_pool,
            tc.tile_pool(name="w8", bufs=4) as w8_pool,
            tc.tile_pool(name="w1", bufs=8) as w1_pool,
            tc.tile_pool(name="w2", bufs=2) as w2_pool,
            tc.tile_pool(name="tg", bufs=3) as tg_pool,
            tc.tile_pool(name="yo", bufs=3) as yo_pool,
            tc.tile_pool(name="ps", bufs=8, space="PSUM") as ps_pool,
        ):
            nch = len(chunks)

            # PE warm-up: dummy matmuls on memset data, issued ahead of the
            # real stream. They run while the head DMAs are in flight, so the
            # PE is at full clock (and HAM at K=8/8) when real data lands,
            # instead of paying the 1.2 GHz ramp + idle-triggered throttle.
            wz = wu_pool.tile([P, P], bf16)
            wx = wu_pool.tile([P, 512], bf16)
            nc.vector.memset(wz[:], 0.0)
            nc.vector.memset(wx[:], 0.0)
            pw = ps_pool.tile([P, 512], f32, tag="ps")
            for _ in range(NWARM):
                nc.tensor.matmul(pw[:], wz[:], wx[:], start=True, stop=True)

            # Head DMA plan, in PE demand order, x spread across both HWDGE
            # rings. The ffn1 unit order below keeps the first three j's on
            # chunk 0 only, so chunk 1 of x isn't needed until ~33us in.
            w1_tiles = {}

            def issue_w1(m, ring):
                t = w1_pool.tile([P, KO, P], bf16, tag="w1s")
                ring.dma_start(t[:], w1_d.ap()[:, m])
                w1_tiles[m] = t

            xt_sb = xt_pool.tile([P, KO, C], bf16)

            def issue_xt(ring, ci, k0, k1):
                c0, cn = chunks[ci]
                ring.dma_start(
                    xt_sb[:, k0:k1, c0 : c0 + cn], xt_d.ap()[:, k0:k1, c0 : c0 + cn]
                )

            t = w1_pool.tile([P, KO, P], bf16, tag="w1s")
            nc.sync.dma_start(t[:, : KO // 2], w1_d.ap()[:, 0, : KO // 2])
            issue_xt(nc.sync, 0, 0, 4)
            issue_xt(nc.scalar, 0, 4, 8)
            nc.sync.dma_start(t[:, KO // 2 :], w1_d.ap()[:, 0, KO // 2 :])
            w1_tiles[0] = t
            issue_xt(nc.scalar, 0, 8, 12)
            t = w1_pool.tile([P, KO, P], bf16, tag="w1s")
            nc.sync.dma_start(t[:, : KO // 2], w1_d.ap()[:, 1, : KO // 2])
            issue_xt(nc.scalar, 0, 12, 16)
            nc.sync.dma_start(t[:, KO // 2 :], w1_d.ap()[:, 1, KO // 2 :])
            w1_tiles[1] = t
            issue_w1(2, nc.sync)
            issue_w1(3, nc.scalar)
            issue_w1(4, nc.scalar)
            issue_w1(5, nc.scalar)
            if nch >= 2:
                issue_xt(nc.sync, 1, 0, 4)
                issue_xt(nc.sync, 1, 4, 8)
                issue_xt(nc.scalar, 1, 8, 12)
                issue_xt(nc.scalar, 1, 12, 16)
            for ci in range(2, nch):
                for k0 in range(0, KO, 4):
                    issue_xt(nc.scalar, ci, k0, k0 + 4)
            x8_sb = x8_pool.tile([P, 2, C], fp8)
            nc.scalar.dma_start(x8_sb[:], x8_d.ap()[:])
            w8_tiles = {}

            def get_w8(j, up=False):
                key = (j, up)
                if key not in w8_tiles:
                    t = w8_pool.tile([P, 2, P], fp8, tag="w8s")
                    nc.scalar.dma_start(
                        t[:], (w8u_d if up else w8_d).ap()[:, :, j]
                    )
                    w8_tiles[key] = t
                return w8_tiles[key]

            act_sb = act_pool.tile([P, MJ, C], bf16)

            # ffn1 unit order: chunk-major over the first three (preloaded)
            # j's so the early PE demand matches the head DMA supply, then
            # j-major for the rest.
            head_js = min(3, MJ)
            order = [(j, ci) for ci in range(nch) for j in range(head_js)]
            order += [(j, ci) for j in range(head_js, MJ) for ci in range(nch)]
            last_unit = {}
            for idx, (j, ci) in enumerate(order):
                last_unit[j] = idx

            def get_w1(m):
                if m not in w1_tiles:
                    t = w1_pool.tile([P, KO, P], bf16, tag="w1s")
                    nc.scalar.dma_start(t[:], w1_d.ap()[:, m])
                    w1_tiles[m] = t
                return w1_tiles[m]

            with nc.named_scope("ffn1"):
                for idx, (j, ci) in enumerate(order):
                    wg = get_w1(2 * j)
                    wu = get_w1(2 * j + 1)
                    c0, cn = chunks[ci]
                    dr = j >= DR_J0
                    pg = ps_pool.tile([P, 512], f32, tag="ps")
                    pu = ps_pool.tile([P, 512], f32, tag="ps")
                    nko_g = KO - 2 if dr else KO
                    for ko in range(nko_g):
                        nc.tensor.matmul(
                            pg[:, :cn],
                            wg[:, ko],
                            xt_sb[:, ko, c0 : c0 + cn],
                            start=(ko == 0),
                            stop=(ko == nko_g - 1) and not dr,
                            skip_group_check=dr,
                        )
                    if dr:
                        nc.tensor.matmul(
                            pg[:, :cn],
                            get_w8(j)[:],
                            x8_sb[:, :, c0 : c0 + cn],
                            start=False,
                            stop=True,
                            perf_mode=DR,
                            skip_group_check=True,
                        )
                    for ko in range(nko_g):
                        nc.tensor.matmul(
                            pu[:, :cn],
                            wu[:, ko],
                            xt_sb[:, ko, c0 : c0 + cn],
                            start=(ko == 0),
                            stop=(ko == nko_g - 1) and not dr,
                            skip_group_check=dr,
                        )
                    if dr:
                        nc.tensor.matmul(
                            pu[:, :cn],
                            get_w8(j, up=True)[:],
                            x8_sb[:, :, c0 : c0 + cn],
                            start=False,
                            stop=True,
                            perf_mode=DR,
                            skip_group_check=True,
                        )
                    tg = tg_pool.tile([P, 512], f32, tag="tg")
                    nc.scalar.activation(tg[:, :cn], pg[:, :cn], act_fn)
                    nc.vector.tensor_mul(
                        out=act_sb[:, j, c0 : c0 + cn],
                        in0=tg[:, :cn],
                        in1=pu[:, :cn],
                    )
                    if idx == last_unit[j]:
                        w1_tiles.pop(2 * j, None)
                        w1_tiles.pop(2 * j + 1, None)
                        w8_tiles.pop((j, False), None)
                        w8_tiles.pop((j, True), None)

            with nc.named_scope("ffn2"):
                for i in range(KO):
                    w2t = w2_pool.tile([P, MJ, P], bf16, tag="w2s")
                    (nc.sync if i < 2 else nc.scalar).dma_start(
                        w2t[:], w2_d.ap()[:, i]
                    )
                    units = list(chunks)
                    if i == KO - 1 and units[-1][1] >= 256:
                        c0, cn = units.pop()
                        h = (cn // 2 + 7) // 8 * 8
                        q = (cn - h) // 2 // 8 * 8
                        units += [(c0, h), (c0 + h, cn - h - q), (c0 + cn - q, q)]
                    for c0, cn in units:
                        py = ps_pool.tile([P, 512], f32, tag="ps")
                        for fo in range(MJ):
                            nc.tensor.matmul(
                                py[:, :cn],
                                w2t[:, fo],
                                act_sb[:, fo, c0 : c0 + cn],
                                start=(fo == 0),
                                stop=(fo == MJ - 1),
                            )
                        yo = yo_pool.tile([P, 512], bf16, tag="yo")
                        nc.vector.tensor_copy(out=yo[:, :cn], in_=py[:, :cn])
                        nc.sync.dma_start(yt_d.ap()[:, i, c0 : c0 + cn], yo[:, :cn])

    nc.compile()
    _BUILD_CACHE[key] = nc
    return nc


def _router(x, router_scale, gate_w):
    """Replicate the reference router ops exactly (same jax ops, default
    backend) so the top-2 expert selection bit-matches the reference."""
    import jax
    import jax.numpy as jnp

    x = jnp.asarray(x)
    router_scale = jnp.asarray(router_scale)
    gate_w = jnp.asarray(gate_w)
    _B, _L, d = x.shape
    h = x * jax.lax.rsqrt(jnp.mean(x * x, axis=-1, keepdims=True) + EPS)
    h = h * (d**-0.5) * router_scale
    logits = (h @ gate_w).astype(jnp.float32)
    probs = jax.nn.softmax(logits, axis=-1)
    w, idx = jax.lax.top_k(probs, TOP_K)
    w = w / jnp.clip(jnp.sum(w, axis=-1, keepdims=True), 1e-12)
    w = w.astype(x.dtype)
    return (
        np.asarray(idx).reshape(-1, TOP_K),
        np.asarray(w).reshape(-1, TOP_K).astype(np.float32),
    )


def _gelu_exact(v: np.ndarray) -> np.ndarray:
    """gelu(approximate=False) = v * Phi(v), matching jax.nn.gelu."""
    try:
        from scipy.special import erf
    except Exception:  # pragma: no cover - scipy ships with jax
        import math

        erf = np.vectorize(math.erf, otypes=[np.float64])
    return (v * 0.5 * (1.0 + erf(v / np.sqrt(2.0)))).astype(np.float32)


def _host_ffn(xe: np.ndarray, gate_up_e: np.ndarray, down_e: np.ndarray):
    """Exact fp32 expert FFN for the few capacity-overflow tokens."""
    h = xe @ gate_up_e
    act = _gelu_exact(h[:, :F]) * h[:, F:]
    return act @ down_e


def _pack_w1(gate_up_e: np.ndarray) -> np.ndarray:
    """[D, 2F] -> [P, 2*MJ, KO, P] with gate/up 128-col strips interleaved."""
    g = gate_up_e[:, :F].reshape(D, MJ, P)
    u = gate_up_e[:, F:].reshape(D, MJ, P)
    w1p = np.empty((D, 2 * MJ, P), np.float32)
    w1p[:, 0::2] = g
    w1p[:, 1::2] = u
    # [D, 2MJ, P] -> [KO, P, 2MJ, P] -> [P, 2MJ, KO, P]
    return np.ascontiguousarray(
        w1p.reshape(KO, P, 2 * MJ, P).transpose(1, 2, 0, 3).astype(BF16)
    )


def _pack_w2(down_e: np.ndarray) -> np.ndarray:
    """[F, D] -> [P, KO, MJ, P]  (w2[p, i, fo, q] = W2[fo*128+p, i*128+q])."""
    return np.ascontiguousarray(
        down_e.reshape(MJ, P, KO, P).transpose(1, 2, 0, 3).astype(BF16)
    )


def run_moe(x, router_scale, gate_w, gate_up, down, per_expert_scale, trace=False):
    from concourse import bass_utils

    x = np.asarray(x, dtype=np.float32)
    router_scale = np.asarray(router_scale, dtype=np.float32)
    gate_w = np.asarray(gate_w, dtype=np.float32)
    gate_up = np.asarray(gate_up, dtype=np.float32)
    down = np.asarray(down, dtype=np.float32)
    per_expert_scale = np.asarray(per_expert_scale, dtype=np.float32)

    B, L, d = x.shape
    N = B * L
    assert d == D and gate_up.shape == (E, D, 2 * F) and down.shape == (E, F, D)

    idxf, wf = _router(x, router_scale, gate_w)

    pair_expert = idxf.reshape(-1)
    pair_token = np.repeat(np.arange(N), TOP_K)
    pair_w = wf.reshape(-1) * per_expert_scale[pair_expert]

    order = np.argsort(pair_expert, kind="stable")
    tok_o = pair_token[order]
    w_o = pair_w[order]
    counts = np.bincount(pair_expert, minlength=E)
    offs = np.zeros(E + 1, np.int64)
    offs[1:] = np.cumsum(counts)

    # Capacity factor 1.0: each core takes up to C tokens of its expert;
    # the <1% overflow of over-subscribed experts is computed exactly on
    # host and merged below.
    CMAX = max(512, -(-(N * TOP_K // E) // 8) * 8)
    C = max(64, min(CMAX, -(-int(counts.max()) // 8) * 8))

    nc = _build(C)

    xf = x.reshape(N, D)
    xf16 = xf.astype(BF16)

    in_maps = []
    dev_n = []
    for e in range(E):
        lo, hi = offs[e], offs[e + 1]
        n_dev = min(C, hi - lo)
        dev_n.append(n_dev)
        toks = tok_o[lo : lo + n_dev]
        xg = np.zeros((C, D), BF16)
        xg[:n_dev] = xf16[toks]
        xt = np.ascontiguousarray(xg.T.reshape(KO, P, C).transpose(1, 0, 2))
        # fp8 pair-packed tails: x8[p,i,c] = X[c, (KO-2+i)*P+p],
        # w8[p,i,j,q] = gate_up[e][(KO-2+i)*P+p, j*P+q] (gate strips only).
        xg8 = np.zeros((C, 2 * P), np.float32)
        xg8[:n_dev] = xf[toks][:, (KO - 2) * P :]
        x8 = np.ascontiguousarray(
            xg8.T.reshape(2, P, C).transpose(1, 0, 2).astype(FP8)
        )
        w8 = np.ascontiguousarray(
            gate_up[e][(KO - 2) * P :, :F]
            .reshape(2, P, MJ, P)
            .transpose(1, 0, 2, 3)
            .astype(FP8)
        )
        w8u = np.ascontiguousarray(
            gate_up[e][(KO - 2) * P :, F:]
            .reshape(2, P, MJ, P)
            .transpose(1, 0, 2, 3)
            .astype(FP8)
        )
        in_maps.append(
            {
                "xt": xt,
                "w1": _pack_w1(gate_up[e]),
                "w2": _pack_w2(down[e]),
                "x8": x8,
                "w8": w8,
                "w8u": w8u,
            }
        )

    res = bass_utils.run_bass_kernel_spmd(
        nc, in_maps, core_ids=list(range(E)), trace=trace
    )

    contrib = np.empty((len(tok_o), D), np.float32)
    for e in range(E):
        lo, hi = offs[e], offs[e + 1]
        n_dev = dev_n[e]
        yt = np.asarray(res.results[e]["yt"]).astype(np.float32)  # [P, KO, C]
        ytd = yt.transpose(1, 0, 2).reshape(D, C)  # [D, C]
        contrib[lo : lo + n_dev] = ytd[:, :n_dev].T
        if n_dev < hi - lo:  # capacity overflow -> exact host FFN
            toks = tok_o[lo + n_dev : hi]
            contrib[lo + n_dev : hi] = _host_ffn(xf[toks], gate_up[e], down[e])

    contrib *= w_o[:, None]

    s = np.argsort(tok_o, kind="stable")
    tok_s = tok_o[s]
    out = np.zeros((N, D), np.float32)
    if len(tok_s) == 2 * N and np.array_equal(tok_s[0::2], tok_s[1::2]):
        cs = contrib[s]
        out[tok_s[0::2]] = cs[0::2] + cs[1::2]
    else:  # defensive fallback (duplicate experts per token can't happen)
        np.add.at(out, tok_o, contrib)
    return out.reshape(B, L, D), res


def kernel(x, router_scale, gate_w, gate_up, down, per_expert_scale):
    out, _ = run_moe(x, router_scale, gate_w, gate_up, down, per_expert_scale)
    return out
